# revision 8
# baseline (speedup 1.0000x reference)
"""Trainium2 Bass kernel for nn_DHHPTransform.

The reference op is: optional stride-2 permutation along N, an upper
tridiagonal Givens sweep, a lower tridiagonal sweep, and a diagonal
scale.  The two sweeps compose into a single *pentadiagonal* operator
  z[i] = sum_{k=-2..2} c_k[i] * x[i+k]
whose coefficients c_k (and the Diag fold) are O(B*N) and precomputed on
host.  The device kernel is then a banded matvec: for each 128-row input
window it runs one fp32 matmul  out[124, 256] = lhsT[128, 124].T @ win
where lhsT holds the 5 coefficient diagonals (host-baked), evicts PSUM
to SBUF, and stores.  Sharding: pure data-parallel, one batch element
per NeuronCore.
"""

import numpy as np

B, N, D = 8, 8192, 256
KWIN = 128           # matmul contraction window (input rows per block)
MOUT = KWIN - 4      # output rows per block (window = out rows +2 halo each side)
NCORES = 8
HALF = N // 2        # even/odd permutation boundary in permuted row space
GH = 11              # blocks per grouped lhsT load / grouped store

_prog_cache = {}


# ---------------------------------------------------------------- host math

def _penta_coeffs(G_l_ii, G_l_ij, G_l_ji, G_l_jj,
                  G_u_ii, G_u_ij, G_u_ji, G_u_jj, Diag, transform):
    """[B, 5, N] pentadiagonal coefficients; index k means offset k-2."""
    Bn, n = Diag.shape
    f8 = np.float64
    u_lo = np.zeros((Bn, n), f8); u_dm = np.zeros((Bn, n), f8); u_hi = np.zeros((Bn, n), f8)
    u_dm[:, 0] = G_u_ii[:, 0]
    u_hi[:, 0] = G_u_ij[:, 0]
    u_lo[:, 1:n-1] = G_u_ji[:, :-1]
    u_dm[:, 1:n-1] = G_u_jj[:, :-1].astype(f8) * G_u_ii[:, 1:]
    u_hi[:, 1:n-1] = G_u_jj[:, :-1].astype(f8) * G_u_ij[:, 1:]
    u_lo[:, n-1] = G_u_ji[:, n-2]
    u_dm[:, n-1] = G_u_jj[:, n-2]
    l_lo = np.zeros((Bn, n), f8); l_dm = np.zeros((Bn, n), f8); l_hi = np.zeros((Bn, n), f8)
    l_dm[:, 0] = G_l_ii[:, 0]
    l_hi[:, 0] = G_l_ij[:, 0]
    l_lo[:, 1:n-1] = G_l_ii[:, 1:n-1].astype(f8) * G_l_ji[:, :n-2]
    l_dm[:, 1:n-1] = G_l_ii[:, 1:n-1].astype(f8) * G_l_jj[:, :n-2]
    l_hi[:, 1:n-1] = G_l_ij[:, 1:n-1]
    l_lo[:, n-1] = G_l_ji[:, n-2]
    l_dm[:, n-1] = G_l_jj[:, n-2]

    def sh(a, k):
        out = np.zeros_like(a)
        if k == 0:
            return a.copy()
        if k > 0:
            out[:, :-k] = a[:, k:]
        else:
            out[:, -k:] = a[:, :k]
        return out

    c = np.zeros((Bn, 5, n), f8)
    c[:, 0] = l_lo * sh(u_lo, -1)
    c[:, 1] = l_lo * sh(u_dm, -1) + l_dm * u_lo
    c[:, 2] = l_lo * sh(u_hi, -1) + l_dm * u_dm + l_hi * sh(u_lo, +1)
    c[:, 3] = l_dm * u_hi + l_hi * sh(u_dm, +1)
    c[:, 4] = l_hi * sh(u_hi, +1)
    c[:, 0, 0:2] = 0
    c[:, 1, 0:1] = 0
    c[:, 3, n-1:] = 0
    c[:, 4, n-2:] = 0
    if transform:
        c *= Diag[:, None, :]
    else:
        for k in range(5):
            c[:, k] = c[:, k] * sh(Diag.astype(f8), k - 2)
    return c


def _block_plan():
    plan = []
    o0 = 0
    while o0 < N:
        mcount = min(MOUT, N - o0)
        w0 = min(max(o0 - 2, 0), N - KWIN)
        plan.append((o0, mcount, w0))
        o0 += mcount
    return plan


def _build_lhst(c, plan, straddle_j):
    """c: [B, 5, N] -> slabs [B, nslot, KWIN, KWIN] fp32 (cols zero-padded).

    Slot j is block j's lhsT.  For the straddle block (transform=1 only) the
    window is split at t = HALF - w0: slot straddle_j keeps rows 0..t-1
    (piece A), slot nblk holds rows t..127 rebased to row 0 (piece B)."""
    nblk = len(plan)
    nslot = nblk + (1 if straddle_j is not None else 0)
    Bn = c.shape[0]
    lhst = np.zeros((Bn, nslot, KWIN, KWIN), np.float32)
    r = np.arange(KWIN)
    for j, (o0, mcount, w0) in enumerate(plan):
        m = np.arange(mcount)
        off = (w0 + r[:, None]) - (o0 + m[None, :])
        valid = (off >= -2) & (off <= 2)
        rr, mm = np.nonzero(valid)
        lhst[:, j, rr, mm] = c[:, off[rr, mm] + 2, o0 + mm].astype(np.float32)
    if straddle_j is not None:
        o0, mcount, w0 = plan[straddle_j]
        t = HALF - w0
        lhst[:, nblk, :KWIN - t, :] = lhst[:, straddle_j, t:, :]
        lhst[:, straddle_j, t:, :] = 0.0
    return lhst


# ---------------------------------------------------------------- device program

def _build_program(transform):
    import concourse.bass as bass
    import concourse.mybir as mybir
    import concourse.tile as tile
    from concourse import bacc

    F32 = mybir.dt.float32
    plan = _block_plan()
    nblk = len(plan)

    straddle_j = None
    if transform:
        for j, (o0, mcount, w0) in enumerate(plan):
            if w0 < HALF < w0 + KWIN:
                straddle_j = j
    nslot = nblk + (1 if straddle_j is not None else 0)

    nc = bacc.Bacc(None, target_bir_lowering=False)
    x = nc.declare_dram_parameter("x", [N, D], F32, isOutput=False)
    lhst = nc.declare_dram_parameter("lhst", [nslot, KWIN, KWIN], F32, isOutput=False)
    z = nc.declare_dram_parameter("z", [N, D], F32, isOutput=True)

    if transform:
        # x_perm[i] = x[2i] (i < HALF) else x[2(i-HALF)+1]
        xv = x.rearrange("(n two) d -> two n d", two=2)

        def win_src(row, cnt):
            # rows row..row+cnt-1 of x_perm, must not cross HALF
            if row + cnt <= HALF:
                return xv[0, row:row + cnt, :]
            assert row >= HALF
            return xv[1, row - HALF:row - HALF + cnt, :]
    else:
        def win_src(row, cnt):
            return x[row:row + cnt, :]

    # store groups: runs of consecutive full (mcount == MOUT) blocks
    groups = []
    jj = 0
    while jj < nblk:
        g = []
        while jj < nblk and plan[jj][1] == MOUT and len(g) < GH:
            g.append(jj)
            jj += 1
        if not g:
            g = [jj]
            jj += 1
        groups.append(g)

    with tile.TileContext(nc) as tc:
        with (
            tc.tile_pool(name="xwin", bufs=4) as xpool,
            tc.tile_pool(name="lh", bufs=2) as lhpool,
            tc.tile_pool(name="psum", bufs=6, space="PSUM") as pspool,
            tc.tile_pool(name="stage", bufs=2) as stpool,
        ):
            ev = 0
            for g in groups:
                glen = len(g)
                # grouped lhsT load (block j -> columns [gi*KWIN, gi*KWIN+KWIN))
                has_straddle = straddle_j in g
                ncols = (glen + 1) if has_straddle else glen
                lht = lhpool.tile([KWIN, ncols * KWIN], F32, tag="lh")
                nc.sync.dma_start(
                    out=lht[:, :glen * KWIN].rearrange("k (j m) -> k j m", m=KWIN),
                    in_=lhst[g[0]:g[0] + glen].rearrange("j k m -> k j m"),
                )
                if has_straddle:
                    nc.sync.dma_start(
                        out=lht[:, glen * KWIN:],
                        in_=lhst[nblk, :, :],
                    )
                full = all(plan[j][1] == MOUT for j in g)
                if full:
                    stg = stpool.tile([MOUT, glen * D], F32, tag="stage")
                for gi, j in enumerate(g):
                    o0, mcount, w0 = plan[j]
                    ps = pspool.tile([mcount, D], F32, tag="psum")
                    lh_ap = lht[:, gi * KWIN: gi * KWIN + mcount]
                    if j == straddle_j:
                        t = HALF - w0
                        xa = xpool.tile([t, D], F32, tag="xa")
                        xb = xpool.tile([KWIN - t, D], F32, tag="xb")
                        nc.sync.dma_start(out=xa[:, :], in_=win_src(w0, t))
                        nc.sync.dma_start(out=xb[:, :], in_=win_src(HALF, KWIN - t))
                        lhb_ap = lht[:, glen * KWIN: glen * KWIN + mcount]
                        nc.tensor.matmul(ps[:, :], lh_ap[:t, :], xa[:, :],
                                         start=True, stop=False)
                        nc.tensor.matmul(ps[:, :], lhb_ap[:KWIN - t, :], xb[:, :],
                                         start=False, stop=True)
                    else:
                        xw = xpool.tile([KWIN, D], F32, tag="xwin")
                        nc.sync.dma_start(out=xw[:, :], in_=win_src(w0, KWIN))
                        nc.tensor.matmul(ps[:, :], lh_ap, xw[:, :],
                                         start=True, stop=True)
                    # PSUM -> SBUF eviction, alternating engines
                    dst = stg[:, gi * D:(gi + 1) * D] if full else None
                    if dst is None:
                        stg1 = stpool.tile([mcount, D], F32, tag="stage_s")
                        dst = stg1[:, :]
                    if ev % 2 == 0:
                        nc.vector.tensor_copy(dst, ps[:, :])
                    else:
                        nc.scalar.copy(dst, ps[:, :])
                    ev += 1
                    if not full:
                        nc.sync.dma_start(out=z[o0:o0 + mcount, :], in_=stg1[:, :])
                if full:
                    o0g = plan[g[0]][0]
                    nc.sync.dma_start(
                        out=z[o0g:o0g + glen * MOUT, :].rearrange(
                            "(g p) d -> p g d", p=MOUT),
                        in_=stg[:, :].rearrange("p (g d) -> p g d", d=D),
                    )
    nc.compile()
    return nc, plan, straddle_j, nslot


def _get_program(transform):
    key = int(bool(transform))
    if key not in _prog_cache:
        _prog_cache[key] = _build_program(key)
    return _prog_cache[key]


# ---------------------------------------------------------------- entry point

def kernel(input, G_l_ii, G_l_ij, G_l_ji, G_l_jj,
           G_u_ii, G_u_ij, G_u_ji, G_u_jj, Diag, transform, _run_kwargs=None):
    from concourse.bass_utils import run_bass_kernel_spmd

    transform = int(np.asarray(transform))
    x_full = np.ascontiguousarray(np.asarray(input, dtype=np.float32))

    nc, plan, straddle_j, nslot = _get_program(transform)
    c = _penta_coeffs(np.asarray(G_l_ii), np.asarray(G_l_ij), np.asarray(G_l_ji),
                      np.asarray(G_l_jj), np.asarray(G_u_ii), np.asarray(G_u_ij),
                      np.asarray(G_u_ji), np.asarray(G_u_jj), np.asarray(Diag),
                      transform)
    lhst = _build_lhst(c, plan, straddle_j)

    in_maps = [
        {"x": x_full[b], "lhst": np.ascontiguousarray(lhst[b])}
        for b in range(B)
    ]
    kw = dict(_run_kwargs or {})
    res = run_bass_kernel_spmd(nc, in_maps, list(range(NCORES)), **kw)
    out = np.stack([res.results[b]["z"] for b in range(B)], axis=0)
    if not transform:
        # store-side stride permutation done on host for the untransformed path
        out = np.concatenate([out[:, 0::2], out[:, 1::2]], axis=1)
    out = out.astype(np.float32, copy=False)
    if _run_kwargs is not None:
        return out, res
    return out


# revision 14
# speedup vs baseline: 89.8385x; 89.8385x over previous
"""Trainium2 Bass kernel for nn_DHHPTransform.

The reference op is: optional stride-2 permutation along N, an upper
tridiagonal Givens sweep, a lower tridiagonal sweep, and a diagonal
scale.  The two sweeps compose into a single *pentadiagonal* operator
  z[i] = sum_{k=-2..2} c_k[i] * x[i+k]
whose coefficients c_k (and the Diag fold) are O(B*N) and precomputed on
host.  The device kernel is then a banded matvec: for each 128-row input
window it runs one fp32 matmul  out[124, 256] = lhsT[128, 124].T @ win
where lhsT holds the 5 coefficient diagonals (host-baked), evicts PSUM
to SBUF, and stores.  Sharding: pure data-parallel, one batch element
per NeuronCore.
"""

import numpy as np

B, N, D = 8, 8192, 256
KWIN = 128           # matmul contraction window (input rows per block)
MOUT = KWIN - 4      # output rows per block (window = out rows +2 halo each side)
NCORES = 8
HALF = N // 2        # even/odd permutation boundary in permuted row space
GH = 11              # blocks per grouped lhsT load / grouped store

_prog_cache = {}


# ---------------------------------------------------------------- host math

def _penta_coeffs(G_l_ii, G_l_ij, G_l_ji, G_l_jj,
                  G_u_ii, G_u_ij, G_u_ji, G_u_jj, Diag, transform):
    """[B, 5, N] pentadiagonal coefficients; index k means offset k-2."""
    Bn, n = Diag.shape
    f8 = np.float64
    u_lo = np.zeros((Bn, n), f8); u_dm = np.zeros((Bn, n), f8); u_hi = np.zeros((Bn, n), f8)
    u_dm[:, 0] = G_u_ii[:, 0]
    u_hi[:, 0] = G_u_ij[:, 0]
    u_lo[:, 1:n-1] = G_u_ji[:, :-1]
    u_dm[:, 1:n-1] = G_u_jj[:, :-1].astype(f8) * G_u_ii[:, 1:]
    u_hi[:, 1:n-1] = G_u_jj[:, :-1].astype(f8) * G_u_ij[:, 1:]
    u_lo[:, n-1] = G_u_ji[:, n-2]
    u_dm[:, n-1] = G_u_jj[:, n-2]
    l_lo = np.zeros((Bn, n), f8); l_dm = np.zeros((Bn, n), f8); l_hi = np.zeros((Bn, n), f8)
    l_dm[:, 0] = G_l_ii[:, 0]
    l_hi[:, 0] = G_l_ij[:, 0]
    l_lo[:, 1:n-1] = G_l_ii[:, 1:n-1].astype(f8) * G_l_ji[:, :n-2]
    l_dm[:, 1:n-1] = G_l_ii[:, 1:n-1].astype(f8) * G_l_jj[:, :n-2]
    l_hi[:, 1:n-1] = G_l_ij[:, 1:n-1]
    l_lo[:, n-1] = G_l_ji[:, n-2]
    l_dm[:, n-1] = G_l_jj[:, n-2]

    def sh(a, k):
        out = np.zeros_like(a)
        if k == 0:
            return a.copy()
        if k > 0:
            out[:, :-k] = a[:, k:]
        else:
            out[:, -k:] = a[:, :k]
        return out

    c = np.zeros((Bn, 5, n), f8)
    c[:, 0] = l_lo * sh(u_lo, -1)
    c[:, 1] = l_lo * sh(u_dm, -1) + l_dm * u_lo
    c[:, 2] = l_lo * sh(u_hi, -1) + l_dm * u_dm + l_hi * sh(u_lo, +1)
    c[:, 3] = l_dm * u_hi + l_hi * sh(u_dm, +1)
    c[:, 4] = l_hi * sh(u_hi, +1)
    c[:, 0, 0:2] = 0
    c[:, 1, 0:1] = 0
    c[:, 3, n-1:] = 0
    c[:, 4, n-2:] = 0
    if transform:
        c *= Diag[:, None, :]
    else:
        for k in range(5):
            c[:, k] = c[:, k] * sh(Diag.astype(f8), k - 2)
    return c


def _block_plan():
    plan = []
    o0 = 0
    while o0 < N:
        mcount = min(MOUT, N - o0)
        w0 = min(max(o0 - 2, 0), N - KWIN)
        plan.append((o0, mcount, w0))
        o0 += mcount
    return plan


def _build_lhst(c, plan, straddle_j):
    """c: [B, 5, N] -> slabs [B, nslot, KWIN, KWIN] fp32 (cols zero-padded).

    Slot j is block j's lhsT.  For the straddle block (transform=1 only) the
    window is split at t = HALF - w0: slot straddle_j keeps rows 0..t-1
    (piece A), slot nblk holds rows t..127 rebased to row 0 (piece B)."""
    nblk = len(plan)
    nslot = nblk + (1 if straddle_j is not None else 0)
    Bn = c.shape[0]
    lhst = np.zeros((Bn, nslot, KWIN, KWIN), np.float32)
    r = np.arange(KWIN)
    for j, (o0, mcount, w0) in enumerate(plan):
        m = np.arange(mcount)
        off = (w0 + r[:, None]) - (o0 + m[None, :])
        valid = (off >= -2) & (off <= 2)
        rr, mm = np.nonzero(valid)
        lhst[:, j, rr, mm] = c[:, off[rr, mm] + 2, o0 + mm].astype(np.float32)
    if straddle_j is not None:
        o0, mcount, w0 = plan[straddle_j]
        t = HALF - w0
        lhst[:, nblk, :KWIN - t, :] = lhst[:, straddle_j, t:, :]
        lhst[:, straddle_j, t:, :] = 0.0
    return lhst


# ---------------------------------------------------------------- device program

def _build_program(transform, reps=1):
    import concourse.bass as bass
    import concourse.mybir as mybir
    import concourse.tile as tile
    from concourse import bacc

    F32 = mybir.dt.float32
    plan = _block_plan()
    nblk = len(plan)

    straddle_j = None
    if transform:
        for j, (o0, mcount, w0) in enumerate(plan):
            if w0 < HALF < w0 + KWIN:
                straddle_j = j
    nslot = nblk + (1 if straddle_j is not None else 0)

    nc = bacc.Bacc(None, target_bir_lowering=False)
    x = nc.declare_dram_parameter("x", [N, D], F32, isOutput=False)
    lhst = nc.declare_dram_parameter("lhst", [nslot, KWIN, KWIN], F32, isOutput=False)
    z = nc.declare_dram_parameter("z", [N, D], F32, isOutput=True)

    from concourse.ap import AP

    def perm_base_step(w0):
        """(element offset, row step) in x for permuted row w0 onward
        (rows must stay within one half for transform=1)."""
        if not transform:
            return w0 * D, D
        if w0 < HALF:
            return 2 * w0 * D, 2 * D
        return (2 * (w0 - HALF) + 1) * D, 2 * D

    def win_src(row, cnt):
        base, step = perm_base_step(row)
        return AP(x, base, [[step, cnt], [1, D]])

    def win_group_src(j0, nwin):
        """One overlapping-window AP [KWIN, nwin, D] for blocks j0..j0+nwin-1."""
        base, step = perm_base_step(plan[j0][2])
        return AP(x, base, [[step, KWIN], [MOUT * step, nwin], [1, D]])

    # x-load chunks: runs of affine same-half windows, split to <= XCH blocks
    XCH = 8
    if transform:
        runs = [[0], list(range(1, straddle_j)), [straddle_j],
                list(range(straddle_j + 1, nblk - 1)), [nblk - 1]]
    else:
        runs = [[0], list(range(1, nblk - 1)), [nblk - 1]]
    xchunks = []
    for r in runs:
        if len(r) == 1:
            xchunks.append(r)
        else:
            for s in range(0, len(r), XCH):
                xchunks.append(r[s:s + XCH])
    xchunk_of = {}
    for ci, chsub in enumerate(xchunks):
        for pos, j in enumerate(chsub):
            xchunk_of[j] = (ci, pos)

    # lhsT chunks of up to LCH slots
    LCH = 17
    lchunk_of = {s: (s // LCH, s % LCH) for s in range(nslot)}
    nlch = (nslot + LCH - 1) // LCH

    # store groups: runs of consecutive full (mcount == MOUT) blocks
    groups = []
    jj = 0
    while jj < nblk:
        g = []
        while jj < nblk and plan[jj][1] == MOUT and len(g) < GH:
            g.append(jj)
            jj += 1
        if not g:
            g = [jj]
            jj += 1
        groups.append(g)

    with tile.TileContext(nc) as tc:
        with (
            tc.tile_pool(name="xg", bufs=3) as xgpool,
            tc.tile_pool(name="xs", bufs=2) as xspool,
            tc.tile_pool(name="lh", bufs=2) as lhpool,
            tc.tile_pool(name="psum", bufs=6, space="PSUM") as pspool,
            tc.tile_pool(name="stage", bufs=2) as stpool,
        ):
            state = {"ev": 0}
            xg_tiles = {}
            lh_tiles = {}

            def ensure_xchunk(ci):
                if ci in xg_tiles:
                    return xg_tiles[ci]
                chsub = xchunks[ci]
                j0 = chsub[0]
                if j0 == straddle_j:
                    t = HALF - plan[j0][2]
                    xa = xspool.tile([t, D], F32, tag="xa")
                    xb = xspool.tile([KWIN - t, D], F32, tag="xb")
                    nc.sync.dma_start(out=xa[:, :], in_=win_src(plan[j0][2], t))
                    nc.sync.dma_start(out=xb[:, :], in_=win_src(HALF, KWIN - t))
                    xg_tiles[ci] = (xa, xb)
                elif len(chsub) == 1:
                    xw = xspool.tile([KWIN, D], F32, tag="xwin")
                    nc.sync.dma_start(out=xw[:, :], in_=win_src(plan[j0][2], KWIN))
                    xg_tiles[ci] = xw
                else:
                    nwin = len(chsub)
                    xt = xgpool.tile([KWIN, nwin * D], F32, tag="xg")
                    nc.sync.dma_start(
                        out=xt[:, :].rearrange("p (j d) -> p j d", d=D),
                        in_=win_group_src(j0, nwin),
                    )
                    xg_tiles[ci] = xt
                return xg_tiles[ci]

            def ensure_lchunk(li):
                if li in lh_tiles:
                    return lh_tiles[li]
                s0 = li * LCH
                cnt = min(LCH, nslot - s0)
                lht = lhpool.tile([KWIN, cnt * KWIN], F32, tag="lh")
                nc.sync.dma_start(
                    out=lht[:, :].rearrange("k (j m) -> k j m", m=KWIN),
                    in_=lhst[s0:s0 + cnt].rearrange("j k m -> k j m"),
                )
                lh_tiles[li] = lht
                return lht

            def emit_body():
                xg_tiles.clear()
                lh_tiles.clear()
                for g in groups:
                    emit_group(g)

            def emit_group(g):
                glen = len(g)
                full = all(plan[j][1] == MOUT for j in g)
                if full:
                    stg = stpool.tile([MOUT, glen * D], F32, tag="stage")
                for gi, j in enumerate(g):
                    o0, mcount, w0 = plan[j]
                    ps = pspool.tile([mcount, D], F32, tag="psum")
                    li, lpos = lchunk_of[j]
                    lht = ensure_lchunk(li)
                    lh_ap = lht[:, lpos * KWIN: lpos * KWIN + mcount]
                    ci, cpos = xchunk_of[j]
                    xt = ensure_xchunk(ci)
                    if j == straddle_j:
                        t = HALF - w0
                        xa, xb = xt
                        lib, lposb = lchunk_of[nblk]
                        lhb = ensure_lchunk(lib)
                        lhb_ap = lhb[:, lposb * KWIN: lposb * KWIN + mcount]
                        nc.tensor.matmul(ps[:, :], lh_ap[:t, :], xa[:, :],
                                         start=True, stop=False)
                        nc.tensor.matmul(ps[:, :], lhb_ap[:KWIN - t, :], xb[:, :],
                                         start=False, stop=True)
                    else:
                        rhs = xt[:, cpos * D:(cpos + 1) * D] if len(xchunks[ci]) > 1 \
                            else xt[:, :]
                        nc.tensor.matmul(ps[:, :], lh_ap, rhs,
                                         start=True, stop=True)
                    # PSUM -> SBUF eviction, mostly DVE (ACT also issues stores)
                    dst = stg[:, gi * D:(gi + 1) * D] if full else None
                    if dst is None:
                        stg1 = stpool.tile([mcount, D], F32, tag="stage_s")
                        dst = stg1[:, :]
                    if state["ev"] % 3 == 2:
                        nc.scalar.copy(dst, ps[:, :])
                    else:
                        nc.vector.tensor_copy(dst, ps[:, :])
                    state["ev"] += 1
                    if not full:
                        nc.scalar.dma_start(out=z[o0:o0 + mcount, :], in_=stg1[:, :])
                if full:
                    o0g = plan[g[0]][0]
                    nc.scalar.dma_start(
                        out=z[o0g:o0g + glen * MOUT, :].rearrange(
                            "(g p) d -> p g d", p=MOUT),
                        in_=stg[:, :].rearrange("p (g d) -> p g d", d=D),
                    )

            if reps == 1:
                emit_body()
            else:
                with tc.For_i(0, reps, 1):
                    emit_body()
    nc.compile()
    return nc, plan, straddle_j, nslot


def _get_program(transform, reps=1):
    key = (int(bool(transform)), reps)
    if key not in _prog_cache:
        _prog_cache[key] = _build_program(key[0], reps)
    return _prog_cache[key]


# ---------------------------------------------------------------- entry point

def kernel(input, G_l_ii, G_l_ij, G_l_ji, G_l_jj,
           G_u_ii, G_u_ij, G_u_ji, G_u_jj, Diag, transform, _run_kwargs=None):
    from concourse.bass_utils import run_bass_kernel_spmd

    transform = int(np.asarray(transform))
    x_full = np.ascontiguousarray(np.asarray(input, dtype=np.float32))

    nc, plan, straddle_j, nslot = _get_program(transform)
    c = _penta_coeffs(np.asarray(G_l_ii), np.asarray(G_l_ij), np.asarray(G_l_ji),
                      np.asarray(G_l_jj), np.asarray(G_u_ii), np.asarray(G_u_ij),
                      np.asarray(G_u_ji), np.asarray(G_u_jj), np.asarray(Diag),
                      transform)
    lhst = _build_lhst(c, plan, straddle_j)

    in_maps = [
        {"x": x_full[b], "lhst": np.ascontiguousarray(lhst[b])}
        for b in range(B)
    ]
    kw = dict(_run_kwargs or {})
    res = run_bass_kernel_spmd(nc, in_maps, list(range(NCORES)), **kw)
    out = np.stack([res.results[b]["z"] for b in range(B)], axis=0)
    if not transform:
        # store-side stride permutation done on host for the untransformed path
        out = np.concatenate([out[:, 0::2], out[:, 1::2]], axis=1)
    out = out.astype(np.float32, copy=False)
    if _run_kwargs is not None:
        return out, res
    return out
